# revision 9
# baseline (speedup 1.0000x reference)
"""Trainium2 Bass kernel for the IMU preintegration module.

Full inputs in, full outputs out; internally data-parallel over 8 NeuronCores
(512 batch rows per core).

Math: the reference scan step is an affine transform on (dr, dv, dp):
    dr' = dr @ R_s,  dv' = dv + dr' (a dt),  dp' = dp + dv' dt + dr' (a dt^2/2)
which composes associatively as T = (A, b, c, n):
    dr' = dr A,  dv' = dv + dr b,  dp' = dp + n dt dv + dr c
with per-step values A_s = R_s, b_s = R_s a_s dt, c_s = (3dt/2) b_s, n_s = 1,
and composition
    A = A1 A2, b = b1 + A1 b2, c = c1 + n2 dt b1 + A1 c2, n = n1 + n2.
Substituting d~ = (c - (3dt/2) b)/dt  gives d~_s = 0 and
    d~ = d~1 + n2 b1 + A1 d~2,
so a left-fold with one step on the right is just:  q <- q (x) q_s,
b <- b + rot(q, a dt), d~ <- d~ + b_old.  Rotations are quaternions; per-step
angles are <= ~0.02 rad so sin/cos/sinc are 2-3 term polynomials in
h = |w dt/2|^2 (no sqrt / division / tables needed).

Each row's 2048 steps = J blocks x K steps: fold K steps vectorized over
(rows x J) lanes, then compose the J block transforms with a binary tree.

Every elementwise op is lane-split between the Vector engine and GpSimd
(disjoint j-ranges, no cross-engine deps); squares/affine go to ScalarE.
"""

import math
import os
import numpy as np

import concourse.mybir as mybir
from concourse import bass, bacc
from concourse.tile import TileContext

F32 = mybir.dt.float32
OP = mybir.AluOpType
AF = mybir.ActivationFunctionType

# problem constants (hardcoded per harness contract)
B_FULL = 4096
S_FULL = 2048
C = 6
N_CORES = 8
R = B_FULL // N_CORES          # rows per core = 512
DT = float(np.float32(1.0 / 200.0))
HALF = DT / 2.0

QSGN = [(1, -1, -1, -1), (1, 1, 1, -1), (1, -1, 1, 1), (1, 1, -1, 1)]
QIDX = [(0, 1, 2, 3), (1, 0, 3, 2), (2, 3, 0, 1), (3, 2, 1, 0)]


def build_nc(rows=R, s_len=S_FULL, j_blocks=128, ksub=4, split_frac=0.61,
             min_split=64):
    nc = bacc.Bacc(None, target_bir_lowering=False, debug=False)
    g = rows // 128
    k = s_len // j_blocks
    assert k % ksub == 0
    x = nc.dram_tensor("x", [rows, s_len, C], F32, kind="ExternalInput")
    out = nc.dram_tensor("out", [rows, 7], F32, kind="ExternalOutput")

    # DRAM view: [g, 128, J, K, C]
    xv = x.rearrange("(g p) (j k) c -> g p j k c", g=g, j=j_blocks)

    def jd_of(n):
        return max(1, min(n - 1, int(round(n * split_frac))))

    # ---- split emission helpers (j = last dim) ----
    def E_tt(o, i0, i1, op):
        n = o.shape[-1]
        if n >= min_split:
            jd = jd_of(n)
            nc.vector.tensor_tensor(out=o[:, :, :jd], in0=i0[:, :, :jd],
                                    in1=i1[:, :, :jd], op=op)
            nc.gpsimd.tensor_tensor(out=o[:, :, jd:], in0=i0[:, :, jd:],
                                    in1=i1[:, :, jd:], op=op)
        else:
            nc.vector.tensor_tensor(out=o, in0=i0, in1=i1, op=op)

    def E_stt(o, i0, sc, i1, op0, op1):
        # TensorScalarPtr is DVE-only on HW (Pool engine rejects it)
        nc.vector.scalar_tensor_tensor(out=o, in0=i0, scalar=sc, in1=i1,
                                       op0=op0, op1=op1)

    def E_ts(o, i0, s1, s2, op0, op1=None):
        nc.vector.tensor_scalar(out=o, in0=i0, scalar1=s1, scalar2=s2,
                                op0=op0, **({"op1": op1} if op1 else {}))

    def qmul_into(nq, q1, q2, tmps):
        """nq = q1 (x) q2 elementwise over lanes. 28 ops."""
        for comp in range(4):
            acc = nq[comp]
            E_tt(acc, q1[0], q2[QIDX[comp][0]], OP.mult)
            for t in range(1, 4):
                tmp = tmps[t % 2]
                E_tt(tmp, q1[t], q2[QIDX[comp][t]], OP.mult)
                E_tt(acc, acc, tmp,
                     OP.add if QSGN[comp][t] > 0 else OP.subtract)

    def cross_into(dst, u, v, tmps):
        """dst = u x v (3-vectors of planes). 9 ops."""
        for i in range(3):
            i1, i2 = (i + 1) % 3, (i + 2) % 3
            E_tt(tmps[0], u[i1], v[i2], OP.mult)
            E_tt(tmps[1], u[i2], v[i1], OP.mult)
            E_tt(dst[i], tmps[0], tmps[1], OP.subtract)

    with TileContext(nc) as tc:
        with (
            tc.tile_pool(name="slab", bufs=2) as slab_pool,
            tc.tile_pool(name="state", bufs=1) as state_pool,
            tc.tile_pool(name="qstate", bufs=2) as q_pool,
            tc.tile_pool(name="cons", bufs=2) as cons_pool,
            tc.tile_pool(name="rots", bufs=1) as rot_pool,
            tc.tile_pool(name="treep", bufs=1) as tree_pool,
        ):
            shp = [128, g, j_blocks]

            # persistent state
            b = [state_pool.tile(shp, F32, tag=f"b{i}", name=f"b{i}")[:]
                 for i in range(3)]
            d = [state_pool.tile(shp, F32, tag=f"d{i}", name=f"d{i}")[:]
                 for i in range(3)]
            for i in range(3):
                nc.vector.memset(d[i], 0.0)

            q = None
            mt = [rot_pool.tile(shp, F32, tag=f"mt{i}", name=f"mt{i}")[:]
                  for i in range(2)]
            cr = [rot_pool.tile(shp, F32, tag=f"cr{i}", name=f"cr{i}")[:]
                  for i in range(3)]
            wr = [rot_pool.tile(shp, F32, tag=f"wr{i}", name=f"wr{i}")[:]
                  for i in range(3)]
            t3 = [rot_pool.tile(shp, F32, tag=f"t3{i}", name=f"t3{i}")[:]
                  for i in range(3)]

            # ---------------- main fold over K steps ----------------
            slab = None
            for s_abs in range(k):
                slab_i, s_loc = divmod(s_abs, ksub)
                if s_loc == 0:
                    slab = slab_pool.tile([128, g, j_blocks, ksub, C], F32,
                                          tag="slab", name="slab")
                    for gi in range(g):
                        # split DMA issue across both HWDGE queues
                        eng = nc.sync if gi % 2 == 0 else nc.scalar
                        eng.dma_start(
                            out=slab[:, gi],
                            in_=xv[gi, :, :, slab_i * ksub:(slab_i + 1) * ksub, :],
                        )
                W = [slab[:, :, :, s_loc, ci] for ci in range(3)]
                A = [slab[:, :, :, s_loc, 3 + ci] for ci in range(3)]

                hx = cons_pool.tile(shp, F32, tag="hx", name="hx")[:]
                hy = cons_pool.tile(shp, F32, tag="hy", name="hy")[:]
                hz = cons_pool.tile(shp, F32, tag="hz", name="hz")[:]
                h = cons_pool.tile(shp, F32, tag="h", name="h")[:]
                qs = [cons_pool.tile(shp, F32, tag=f"qs{i}", name=f"qs{i}")[:]
                      for i in range(4)]

                nc.scalar.activation(hx, W[0], AF.Square, scale=HALF)
                nc.scalar.activation(hy, W[1], AF.Square, scale=HALF)
                nc.scalar.activation(hz, W[2], AF.Square, scale=HALF)
                E_tt(h, hx, hy, OP.add)
                E_tt(h, h, hz, OP.add)
                # qsw = 1 + (h^2/24 - h/2)
                E_stt(hy, h, 1.0 / 24.0, h, OP.mult, OP.mult)
                E_stt(hz, h, -0.5, hy, OP.mult, OP.add)
                nc.scalar.activation(qs[0], hz, AF.Identity, bias=1.0)
                # p1 = 1 + (h^2/120 - h/6);  qs_v = (dt/2) * p1 * w
                E_stt(hy, h, 1.0 / 120.0, h, OP.mult, OP.mult)
                E_stt(hz, h, -1.0 / 6.0, hy, OP.mult, OP.add)
                nc.scalar.activation(hx, hz, AF.Identity, bias=1.0)
                for i in range(3):
                    E_stt(qs[1 + i], W[i], HALF, hx, OP.mult, OP.mult)

                # orientation chain
                if s_abs == 0:
                    q = qs
                else:
                    nq = [q_pool.tile(shp, F32, tag=f"q{i}", name=f"q{i}")[:]
                          for i in range(4)]
                    qmul_into(nq, q, qs, mt)
                    q = nq

                # d~ += b_old (before b update below)
                if s_abs >= 1:
                    for i in range(3):
                        E_tt(d[i], d[i], b[i], OP.add)

                # rot(q, a) = a + 2 qv x (qv x a + qw a)
                # b <- b + dt*a + 2*dt*(qv x (qv x a + qw a))
                qv = q[1:]
                cross_into(cr, qv, A, mt)
                for i in range(3):
                    E_tt(wr[i], q[0], A[i], OP.mult)
                for i in range(3):
                    E_tt(cr[i], cr[i], wr[i], OP.add)
                cross_into(wr, qv, cr, mt)
                if s_abs == 0:
                    for i in range(3):
                        E_ts(t3[i], A[i], DT, None, OP.mult)
                else:
                    for i in range(3):
                        E_stt(t3[i], A[i], DT, b[i], OP.mult, OP.add)
                for i in range(3):
                    E_stt(b[i], wr[i], 2.0 * DT, t3[i], OP.mult, OP.add)

            # ---------------- binary tree over J blocks ----------------
            n_lvl = int(math.log2(j_blocks))
            n2 = float(k)
            cur_q = q
            for lvl in range(1, n_lvl + 1):
                step = 1 << lvl
                half = step >> 1
                n = j_blocks >> lvl
                lshp = [128, g, n]
                q1 = [t[:, :, 0::2] for t in cur_q]
                q2 = [t[:, :, 1::2] for t in cur_q]
                bL = [t[:, :, 0::step] for t in b]
                bR = [t[:, :, half::step] for t in b]
                dL = [t[:, :, 0::step] for t in d]
                dR = [t[:, :, half::step] for t in d]

                nq = [tree_pool.tile(lshp, F32, tag=f"tq{lvl}_{i}",
                                     name=f"tq{lvl}_{i}")[:] for i in range(4)]
                tc_c = [tree_pool.tile(lshp, F32, tag=f"tc{i}",
                                       name=f"tc{i}")[:] for i in range(3)]
                tc_w = [tree_pool.tile(lshp, F32, tag=f"tw{i}",
                                       name=f"tw{i}")[:] for i in range(3)]
                tm = [tree_pool.tile(lshp, F32, tag=f"tm{i}",
                                     name=f"tm{i}")[:] for i in range(2)]

                qmul_into(nq, q1, q2, tm)
                qv1 = q1[1:]

                # rot(q1, d2); d1 += n2*b1 + d2 + 2*p  (reads b1 BEFORE b update)
                cross_into(tc_c, qv1, dR, tm)
                for i in range(3):
                    E_tt(tc_w[i], q1[0], dR[i], OP.mult)
                for i in range(3):
                    E_tt(tc_c[i], tc_c[i], tc_w[i], OP.add)
                cross_into(tc_w, qv1, tc_c, tm)
                for i in range(3):
                    E_stt(dL[i], bL[i], n2, dL[i], OP.mult, OP.add)
                for i in range(3):
                    E_tt(dL[i], dL[i], dR[i], OP.add)
                for i in range(3):
                    E_stt(dL[i], tc_w[i], 2.0, dL[i], OP.mult, OP.add)

                # rot(q1, b2); b1 += b2 + 2*p
                cross_into(tc_c, qv1, bR, tm)
                for i in range(3):
                    E_tt(tc_w[i], q1[0], bR[i], OP.mult)
                for i in range(3):
                    E_tt(tc_c[i], tc_c[i], tc_w[i], OP.add)
                cross_into(tc_w, qv1, tc_c, tm)
                for i in range(3):
                    E_tt(bL[i], bL[i], bR[i], OP.add)
                for i in range(3):
                    E_stt(bL[i], tc_w[i], 2.0, bL[i], OP.mult, OP.add)

                cur_q = nq
                n2 *= 2.0

            # ---------------- finalize ----------------
            bF = [t[:, :, 0:1] for t in b]
            dF = [t[:, :, 0:1] for t in d]
            out_t = state_pool.tile([128, g, 7], F32, tag="outt", name="outt")[:]
            tf = state_pool.tile([128, g, 1], F32, tag="tf", name="tf")[:]
            for i in range(3):
                nc.vector.scalar_tensor_tensor(
                    out=tf, in0=bF[i], scalar=1.5, in1=dF[i],
                    op0=OP.mult, op1=OP.add)
                nc.vector.tensor_scalar(
                    out=out_t[:, :, i:i + 1], in0=tf, scalar1=DT, scalar2=None,
                    op0=OP.mult)
            sg = state_pool.tile([128, g, 1], F32, tag="sg", name="sg")[:]
            nc.vector.tensor_scalar(
                out=sg, in0=cur_q[0], scalar1=0.0, scalar2=2.0,
                op0=OP.is_ge, op1=OP.mult)
            nc.vector.tensor_scalar(
                out=sg, in0=sg, scalar1=-1.0, scalar2=None, op0=OP.add)
            for i in range(4):
                nc.vector.tensor_tensor(
                    out=out_t[:, :, 3 + i:4 + i], in0=cur_q[i], in1=sg,
                    op=OP.mult)

            ov = out.rearrange("(g p) c -> g p c", g=g)
            for gi in range(g):
                nc.sync.dma_start(out=ov[gi], in_=out_t[:, gi, :])

    nc.compile()
    return nc


_NC_CACHE = {}
LAST_RESULTS = None


def _ensure_profiling_hooks():
    """Best-effort: provide the antenv.axon_hooks shim + skip S3 upload so
    trace=True works in this stripped container. No-op on failure."""
    import sys
    import types
    try:
        if "antenv.axon_hooks" not in sys.modules:
            from trn_agent_boot.trn_boot import _ntff_profile_via_ctypes
            hook = _ntff_profile_via_ctypes("/opt/axon/libaxon_pjrt.so")
            mod = types.ModuleType("antenv.axon_hooks")
            mod._hook = hook
            mod.get_axon_ntff_profile_hook = lambda: mod._hook
            mod.set_axon_ntff_profile_hook = lambda h: setattr(mod, "_hook", h)
            sys.modules["antenv.axon_hooks"] = mod
        import concourse.bass_utils as bu
        bu.upload_artifacts = lambda tmpdir: tmpdir
    except Exception as e:  # pragma: no cover
        print(f"profiling hook setup failed ({e}); tracing may be skipped")


def kernel(input_seq: np.ndarray) -> np.ndarray:
    from concourse.bass_utils import run_bass_kernel_spmd

    global LAST_RESULTS
    input_seq = np.ascontiguousarray(np.asarray(input_seq, dtype=np.float32))
    assert input_seq.shape == (B_FULL, S_FULL, C), input_seq.shape

    if "nc" not in _NC_CACHE:
        _NC_CACHE["nc"] = build_nc()
    nc = _NC_CACHE["nc"]

    in_maps = [{"x": input_seq[i * R:(i + 1) * R]} for i in range(N_CORES)]
    trace = os.environ.get("BASS_KERNEL_TRACE", "0") == "1"
    if trace:
        _ensure_profiling_hooks()
    res = run_bass_kernel_spmd(nc, in_maps, core_ids=list(range(N_CORES)),
                               trace=trace)
    LAST_RESULTS = res
    return np.concatenate([r["out"] for r in res.results], axis=0)


# revision 11
# speedup vs baseline: 1.3141x; 1.3141x over previous
"""Trainium2 Bass kernel for the IMU preintegration module.

Full inputs in, full outputs out; internally data-parallel over 8 NeuronCores
(512 batch rows per core).

Math: the reference scan step is an affine transform on (dr, dv, dp):
    dr' = dr @ R_s,  dv' = dv + dr' (a dt),  dp' = dp + dv' dt + dr' (a dt^2/2)
which composes associatively as T = (A, b, c, n):
    dr' = dr A,  dv' = dv + dr b,  dp' = dp + n dt dv + dr c
with per-step values A_s = R_s, b_s = R_s a_s dt, c_s = (3dt/2) b_s, n_s = 1,
and composition
    A = A1 A2, b = b1 + A1 b2, c = c1 + n2 dt b1 + A1 c2, n = n1 + n2.
Substituting d~ = (c - (3dt/2) b)/dt  gives d~_s = 0 and
    d~ = d~1 + n2 b1 + A1 d~2,
so a left-fold with one step on the right is just:  q <- q (x) q_s,
b <- b + rot(q, a dt), d~ <- d~ + b_old.  Rotations are quaternions; per-step
angles are <= ~0.02 rad so sin/cos/sinc are 2-3 term polynomials in
h = |w dt/2|^2 (no sqrt / division / tables needed).

Each row's 2048 steps = J blocks x K steps: fold K steps vectorized over
(rows x J) lanes, then compose the J block transforms with a binary tree
(dense compaction every level).

Engine split: ScalarE deinterleaves/scales the strided inputs into dense
planes (strided SBUF reads cost ~3x on DVE) and evaluates squares/affine;
GpSimd takes the dense running-sum adds; everything else is DVE, all dense.
"""

import math
import os
import numpy as np

import concourse.mybir as mybir
from concourse import bass, bacc
from concourse.tile import TileContext

F32 = mybir.dt.float32
OP = mybir.AluOpType
AF = mybir.ActivationFunctionType

# problem constants (hardcoded per harness contract)
B_FULL = 4096
S_FULL = 2048
C = 6
N_CORES = 8
R = B_FULL // N_CORES          # rows per core = 512
DT = float(np.float32(1.0 / 200.0))
HALF = DT / 2.0

QSGN = [(1, -1, -1, -1), (1, 1, 1, -1), (1, -1, 1, 1), (1, 1, -1, 1)]
QIDX = [(0, 1, 2, 3), (1, 0, 3, 2), (2, 3, 0, 1), (3, 2, 1, 0)]


def build_nc(rows=R, s_len=S_FULL, j_blocks=128, ksub=4):
    nc = bacc.Bacc(None, target_bir_lowering=False, debug=False)
    g = rows // 128
    k = s_len // j_blocks
    assert k % ksub == 0
    x = nc.dram_tensor("x", [rows, s_len, C], F32, kind="ExternalInput")
    out = nc.dram_tensor("out", [rows, 7], F32, kind="ExternalOutput")

    # DRAM view: [g, 128, J, K, C]
    xv = x.rearrange("(g p) (j k) c -> g p j k c", g=g, j=j_blocks)

    V = nc.vector
    G = nc.gpsimd

    def qmul_into(nq, q1, q2, tmps):
        """nq = q1 (x) q2 elementwise over lanes. 28 DVE ops."""
        for comp in range(4):
            acc = nq[comp]
            V.tensor_tensor(out=acc, in0=q1[0], in1=q2[QIDX[comp][0]], op=OP.mult)
            for t in range(1, 4):
                tmp = tmps[t % 2]
                V.tensor_tensor(out=tmp, in0=q1[t], in1=q2[QIDX[comp][t]], op=OP.mult)
                V.tensor_tensor(out=acc, in0=acc, in1=tmp,
                                op=OP.add if QSGN[comp][t] > 0 else OP.subtract)

    def cross_into(dst, u, v, tmps):
        """dst = u x v (3-vectors of planes). 9 DVE ops."""
        for i in range(3):
            i1, i2 = (i + 1) % 3, (i + 2) % 3
            V.tensor_tensor(out=tmps[0], in0=u[i1], in1=v[i2], op=OP.mult)
            V.tensor_tensor(out=tmps[1], in0=u[i2], in1=v[i1], op=OP.mult)
            V.tensor_tensor(out=dst[i], in0=tmps[0], in1=tmps[1], op=OP.subtract)

    with TileContext(nc) as tc:
        with (
            tc.tile_pool(name="slab", bufs=2) as slab_pool,
            tc.tile_pool(name="state", bufs=1) as state_pool,
            tc.tile_pool(name="qstate", bufs=2) as q_pool,
            tc.tile_pool(name="cons", bufs=2) as cons_pool,
            tc.tile_pool(name="rots", bufs=1) as rot_pool,
            tc.tile_pool(name="treep", bufs=1) as tree_pool,
        ):
            shp = [128, g, j_blocks]

            b = [state_pool.tile(shp, F32, tag=f"b{i}", name=f"b{i}")[:]
                 for i in range(3)]
            d = [state_pool.tile(shp, F32, tag=f"d{i}", name=f"d{i}")[:]
                 for i in range(3)]
            for i in range(3):
                V.memset(d[i], 0.0)

            q = None
            mt = [rot_pool.tile(shp, F32, tag=f"mt{i}", name=f"mt{i}")[:]
                  for i in range(2)]
            cr = [rot_pool.tile(shp, F32, tag=f"cr{i}", name=f"cr{i}")[:]
                  for i in range(3)]
            wr = [rot_pool.tile(shp, F32, tag=f"wr{i}", name=f"wr{i}")[:]
                  for i in range(3)]
            t3 = cr  # cr is dead after the second cross; reuse as t3

            # ---------------- main fold over K steps ----------------
            slab = None
            for s_abs in range(k):
                slab_i, s_loc = divmod(s_abs, ksub)
                if s_loc == 0:
                    slab = slab_pool.tile([128, g, j_blocks, ksub, C], F32,
                                          tag="slab", name="slab")
                    for gi in range(g):
                        eng = nc.sync if gi % 2 == 0 else nc.scalar
                        eng.dma_start(
                            out=slab[:, gi],
                            in_=xv[gi, :, :, slab_i * ksub:(slab_i + 1) * ksub, :],
                        )
                W = [slab[:, :, :, s_loc, ci] for ci in range(3)]
                A = [slab[:, :, :, s_loc, 3 + ci] for ci in range(3)]

                hx = cons_pool.tile(shp, F32, tag="hx", name="hx")[:]
                hy = cons_pool.tile(shp, F32, tag="hy", name="hy")[:]
                hz = cons_pool.tile(shp, F32, tag="hz", name="hz")[:]
                h = cons_pool.tile(shp, F32, tag="h", name="h")[:]
                qs0 = cons_pool.tile(shp, F32, tag="qs0", name="qs0")[:]
                # wd = (dt/2) w  -> becomes qs vector part in place
                wd = [cons_pool.tile(shp, F32, tag=f"wd{i}", name=f"wd{i}")[:]
                      for i in range(3)]
                # v = dt * a (dense)
                v = [cons_pool.tile(shp, F32, tag=f"v{i}", name=f"v{i}",
                                    bufs=1)[:] for i in range(3)]

                # ScalarE: squares (strided reads), dense scaled copies
                nc.scalar.activation(hx, W[0], AF.Square, scale=HALF)
                nc.scalar.activation(hy, W[1], AF.Square, scale=HALF)
                nc.scalar.activation(hz, W[2], AF.Square, scale=HALF)
                for i in range(3):
                    nc.scalar.mul(wd[i], W[i], HALF)
                for i in range(3):
                    nc.scalar.mul(v[i], A[i], DT)

                # GpSimd: h = hx + hy + hz (dense)
                G.tensor_tensor(out=h, in0=hx, in1=hy, op=OP.add)
                G.tensor_tensor(out=h, in0=h, in1=hz, op=OP.add)

                # qsw = 1 + (h^2/24 - h/2)
                V.scalar_tensor_tensor(out=hy, in0=h, scalar=1.0 / 24.0, in1=h,
                                       op0=OP.mult, op1=OP.mult)
                V.scalar_tensor_tensor(out=hz, in0=h, scalar=-0.5, in1=hy,
                                       op0=OP.mult, op1=OP.add)
                nc.scalar.activation(qs0, hz, AF.Identity, bias=1.0)
                # p1 = 1 + (h^2/120 - h/6);  qs_v = p1 * wd  (in place on wd)
                V.scalar_tensor_tensor(out=hy, in0=h, scalar=1.0 / 120.0, in1=h,
                                       op0=OP.mult, op1=OP.mult)
                V.scalar_tensor_tensor(out=hz, in0=h, scalar=-1.0 / 6.0, in1=hy,
                                       op0=OP.mult, op1=OP.add)
                nc.scalar.activation(hx, hz, AF.Identity, bias=1.0)
                for i in range(3):
                    V.tensor_tensor(out=wd[i], in0=wd[i], in1=hx, op=OP.mult)
                qs = [qs0, wd[0], wd[1], wd[2]]

                # orientation chain
                if s_abs == 0:
                    q = qs
                else:
                    nq = [q_pool.tile(shp, F32, tag=f"q{i}", name=f"q{i}")[:]
                          for i in range(4)]
                    qmul_into(nq, q, qs, mt)
                    q = nq

                # d~ += b_old (GpSimd; before b update below)
                if s_abs >= 1:
                    for i in range(3):
                        G.tensor_tensor(out=d[i], in0=d[i], in1=b[i], op=OP.add)

                # rot(q, v) = v + 2 qv x (qv x v + qw v);  b += rot(q, v)
                qv = q[1:]
                cross_into(cr, qv, v, mt)
                for i in range(3):
                    V.tensor_tensor(out=wr[i], in0=q[0], in1=v[i], op=OP.mult)
                for i in range(3):
                    V.tensor_tensor(out=cr[i], in0=cr[i], in1=wr[i], op=OP.add)
                cross_into(wr, qv, cr, mt)
                if s_abs == 0:
                    for i in range(3):
                        V.scalar_tensor_tensor(out=b[i], in0=wr[i], scalar=2.0,
                                               in1=v[i], op0=OP.mult, op1=OP.add)
                else:
                    for i in range(3):
                        V.tensor_tensor(out=t3[i], in0=v[i], in1=b[i], op=OP.add)
                    for i in range(3):
                        V.scalar_tensor_tensor(out=b[i], in0=wr[i], scalar=2.0,
                                               in1=t3[i], op0=OP.mult, op1=OP.add)

            # ------------- binary tree over J blocks (dense compaction) -------------
            n_lvl = int(math.log2(j_blocks))
            n2 = float(k)
            cur_q, cur_b, cur_d = q, b, d
            for lvl in range(1, n_lvl + 1):
                n = j_blocks >> lvl
                lshp = [128, g, n]
                q1 = [t[:, :, 0::2] for t in cur_q]
                q2 = [t[:, :, 1::2] for t in cur_q]
                b1 = [t[:, :, 0::2] for t in cur_b]
                b2 = [t[:, :, 1::2] for t in cur_b]
                d1 = [t[:, :, 0::2] for t in cur_d]
                d2 = [t[:, :, 1::2] for t in cur_d]

                nq = [tree_pool.tile(lshp, F32, tag=f"tq{lvl}_{i}",
                                     name=f"tq{lvl}_{i}")[:] for i in range(4)]
                nb = [tree_pool.tile(lshp, F32, tag=f"tb{lvl}_{i}",
                                     name=f"tb{lvl}_{i}")[:] for i in range(3)]
                nd = [tree_pool.tile(lshp, F32, tag=f"td{lvl}_{i}",
                                     name=f"td{lvl}_{i}")[:] for i in range(3)]
                tc_c = [tree_pool.tile(lshp, F32, tag=f"tc{i}",
                                       name=f"tc{i}")[:] for i in range(3)]
                tc_w = [tree_pool.tile(lshp, F32, tag=f"tw{i}",
                                       name=f"tw{i}")[:] for i in range(3)]
                tm = [tree_pool.tile(lshp, F32, tag=f"tm{i}",
                                     name=f"tm{i}")[:] for i in range(2)]

                qmul_into(nq, q1, q2, tm)
                qv1 = q1[1:]

                # nd = d1 + n2*b1 + rot(q1, d2)
                cross_into(tc_c, qv1, d2, tm)
                for i in range(3):
                    V.tensor_tensor(out=tc_w[i], in0=q1[0], in1=d2[i], op=OP.mult)
                for i in range(3):
                    V.tensor_tensor(out=tc_c[i], in0=tc_c[i], in1=tc_w[i], op=OP.add)
                cross_into(tc_w, qv1, tc_c, tm)
                for i in range(3):
                    V.scalar_tensor_tensor(out=nd[i], in0=b1[i], scalar=n2,
                                           in1=d1[i], op0=OP.mult, op1=OP.add)
                for i in range(3):
                    G.tensor_tensor(out=nd[i], in0=nd[i], in1=d2[i], op=OP.add)
                for i in range(3):
                    V.scalar_tensor_tensor(out=nd[i], in0=tc_w[i], scalar=2.0,
                                           in1=nd[i], op0=OP.mult, op1=OP.add)

                # nb = b1 + rot(q1, b2)
                cross_into(tc_c, qv1, b2, tm)
                for i in range(3):
                    V.tensor_tensor(out=tc_w[i], in0=q1[0], in1=b2[i], op=OP.mult)
                for i in range(3):
                    V.tensor_tensor(out=tc_c[i], in0=tc_c[i], in1=tc_w[i], op=OP.add)
                cross_into(tc_w, qv1, tc_c, tm)
                for i in range(3):
                    G.tensor_tensor(out=nb[i], in0=b1[i], in1=b2[i], op=OP.add)
                for i in range(3):
                    V.scalar_tensor_tensor(out=nb[i], in0=tc_w[i], scalar=2.0,
                                           in1=nb[i], op0=OP.mult, op1=OP.add)

                cur_q, cur_b, cur_d = nq, nb, nd
                n2 *= 2.0

            # ---------------- finalize ----------------
            out_t = state_pool.tile([128, g, 7], F32, tag="outt", name="outt")[:]
            tf = state_pool.tile([128, g, 1], F32, tag="tf", name="tf")[:]
            for i in range(3):
                V.scalar_tensor_tensor(out=tf, in0=cur_b[i], scalar=1.5,
                                       in1=cur_d[i], op0=OP.mult, op1=OP.add)
                V.tensor_scalar(out=out_t[:, :, i:i + 1], in0=tf, scalar1=DT,
                                scalar2=None, op0=OP.mult)
            sg = state_pool.tile([128, g, 1], F32, tag="sg", name="sg")[:]
            V.tensor_scalar(out=sg, in0=cur_q[0], scalar1=0.0, scalar2=2.0,
                            op0=OP.is_ge, op1=OP.mult)
            V.tensor_scalar(out=sg, in0=sg, scalar1=-1.0, scalar2=None, op0=OP.add)
            for i in range(4):
                V.tensor_tensor(out=out_t[:, :, 3 + i:4 + i], in0=cur_q[i],
                                in1=sg, op=OP.mult)

            ov = out.rearrange("(g p) c -> g p c", g=g)
            for gi in range(g):
                nc.sync.dma_start(out=ov[gi], in_=out_t[:, gi, :])

    nc.compile()
    return nc


_NC_CACHE = {}
LAST_RESULTS = None


def _ensure_profiling_hooks():
    """Best-effort: provide the antenv.axon_hooks shim + skip S3 upload so
    trace=True works in this stripped container. No-op on failure."""
    import sys
    import types
    try:
        if "antenv.axon_hooks" not in sys.modules:
            from trn_agent_boot.trn_boot import _ntff_profile_via_ctypes
            hook = _ntff_profile_via_ctypes("/opt/axon/libaxon_pjrt.so")
            mod = types.ModuleType("antenv.axon_hooks")
            mod._hook = hook
            mod.get_axon_ntff_profile_hook = lambda: mod._hook
            mod.set_axon_ntff_profile_hook = lambda h: setattr(mod, "_hook", h)
            sys.modules["antenv.axon_hooks"] = mod
        import concourse.bass_utils as bu
        bu.upload_artifacts = lambda tmpdir: tmpdir
    except Exception as e:  # pragma: no cover
        print(f"profiling hook setup failed ({e}); tracing may be skipped")


def kernel(input_seq: np.ndarray) -> np.ndarray:
    from concourse.bass_utils import run_bass_kernel_spmd

    global LAST_RESULTS
    input_seq = np.ascontiguousarray(np.asarray(input_seq, dtype=np.float32))
    assert input_seq.shape == (B_FULL, S_FULL, C), input_seq.shape

    if "nc" not in _NC_CACHE:
        _NC_CACHE["nc"] = build_nc()
    nc = _NC_CACHE["nc"]

    in_maps = [{"x": input_seq[i * R:(i + 1) * R]} for i in range(N_CORES)]
    trace = os.environ.get("BASS_KERNEL_TRACE", "0") == "1"
    if trace:
        _ensure_profiling_hooks()
    res = run_bass_kernel_spmd(nc, in_maps, core_ids=list(range(N_CORES)),
                               trace=trace)
    LAST_RESULTS = res
    return np.concatenate([r["out"] for r in res.results], axis=0)
